# revision 45
# baseline (speedup 1.0000x reference)
"""Multi-head attention (16 heads, RoPE, causal) Trainium2 Bass kernel.

Sharding: 8 cores = 4-way data-parallel over batch x 2-way tensor-parallel
over heads (each core: 1 batch, 8 heads). Per-core partial outputs (over its
8 heads) are summed pairwise on the host (the w_o "all-reduce").

v1 design (all-bf16 matmul path):
  - All matmul operands bf16 (stationaries get Fast Weight Load, no fp32r
    narrow-moving penalty, half SBUF traffic). PSUM accumulation is fp32.
  - x passed pre-transposed xT [e, s]; per-head w blocks [e, d] so
    projections produce qT/kT [d, s] directly.
  - RoPE: qrot = C (.) q + S' (.) qp with qp from pair-permuted weight
    copies; the two multiplies run on DVE (PSUM fp32 in, bf16 out), the
    add runs on DVE in bf16 2x mode (one [P,1024] instr per tensor).
  - S^T[k, q] per k-tile j lands in 2-bank [P, <=1024] PSUM tiles grouped
    so ONE Scalar-engine exp covers each tile (6 exps/head instead of 12):
    tiles j=0..3 alone, (j=4,5) share, (j=6,7) share. Causal diagonal gets
    a -1e30 upper-tri bias via one bf16 idn x tri matmul per j.
  - P^T stored PACKED by triangle: pT[:, off_j + (q - j*128)] with
    off_j = j*1024 - 64*j*(j-1), total 4608 cols (bf16).
  - rowsums: 12 ones-matmuls (same stationary back to back) into one
    [P, 1024] PSUM tile; AV into another; normalization = single GPSIMD
    divide per head ([P,1024], PSUM/PSUM -> bf16 SBUF) - no reciprocal.
  - v for all heads projected once (x-tile stationary), copied PSUM->SBUF
    bf16 on GPSIMD.
  - out^T accumulated over heads in 2 persistent PSUM banks, DMA'd
    directly PSUM->HBM.
"""

import os
import sys

import ml_dtypes
import numpy as np

for _p in ("/opt/trn_rl_repo",):
    if os.path.isdir(_p) and _p not in sys.path:
        sys.path.append(_p)

import concourse.bass as bass  # noqa: E402
import concourse.tile as tile  # noqa: E402
from concourse import bacc, mybir  # noqa: E402
import concourse.bass_utils as _bass_utils  # noqa: E402
from concourse.bass_utils import run_bass_kernel_spmd  # noqa: E402



F32 = mybir.dt.float32
BF16 = mybir.dt.bfloat16

B, S, E, H = 4, 1024, 128, 16
NCORES = 8
NH = 8          # heads per core
P = 128
NT = S // P     # 8 seq tiles
SCALE = 1.0 / float(np.sqrt(np.float32(E)))
Exp = mybir.ActivationFunctionType.Exp
MULT = mybir.AluOpType.mult
ADD = mybir.AluOpType.add
DIV = mybir.AluOpType.divide

# packed-triangle column offsets: off[j] = sum_{i<j} (S - i*P)
OFF = [0]
for _j in range(NT):
    OFF.append(OFF[-1] + S - _j * P)
PTW = OFF[NT]   # 4608 packed columns per head


def build_bass():
    nc = bacc.Bacc("TRN2", target_bir_lowering=False, debug=False,
                   num_devices=NCORES)

    def din(name, shape, dt=BF16):
        return nc.dram_tensor(name, shape, dt, kind="ExternalInput").ap()

    xT = din("xT", [P, S])
    wqT = din("wqT", [P, NH * P])
    wkT = din("wkT", [P, NH * P])
    wvT = din("wvT", [P, NH * P])
    woT = din("woT", [P, NH * P])
    ropeC = din("ropeC", [P, S], F32)
    ropeS2 = din("ropeS2", [P, S], F32)
    tri = din("tri", [P, P])
    idn = din("idn", [P, P])
    ones = din("ones", [P, P])
    outT = nc.dram_tensor("outT", [P, S], F32, kind="ExternalOutput").ap()

    with tile.TileContext(nc) as tc:
        _build(tc, xT, wqT, wkT, wvT, woT, ropeC, ropeS2, tri,
               idn, ones, outT)
    nc.compile()
    return nc


def _build(tc, xT, wqT, wkT, wvT, woT, ropeC, ropeS2, tri, idn,
           ones, outT):
    nc = tc.nc

    from contextlib import ExitStack
    ctx = ExitStack()
    const = ctx.enter_context(tc.tile_pool(name="const", bufs=1))
    vpool = ctx.enter_context(tc.tile_pool(name="vpool", bufs=1))
    ppool = ctx.enter_context(tc.tile_pool(name="ppool", bufs=3))
    qkpool = ctx.enter_context(tc.tile_pool(name="qkpool", bufs=6))
    tmppool = ctx.enter_context(tc.tile_pool(name="tmppool", bufs=3))
    stgpool = ctx.enter_context(tc.tile_pool(name="stgpool", bufs=3))
    npool = ctx.enter_context(tc.tile_pool(name="npool", bufs=2))
    pp = ctx.enter_context(tc.tile_pool(name="pp", bufs=2, space="PSUM"))
    sp = ctx.enter_context(tc.tile_pool(name="sp", bufs=2, space="PSUM"))
    op = ctx.enter_context(tc.tile_pool(name="op", bufs=2, space="PSUM"))

    # ---- constants into SBUF in first-use order, spread across the idle
    # engine DMA queues so issue time (~0.6us each) isn't serialized on SP
    def load(pool, ap, shape, dt, tag, eng=None):
        t = pool.tile(shape, dt, tag=tag)
        (eng or nc.sync).dma_start(t[:], ap)
        return t

    xT_sb = load(const, xT, [P, S], BF16, "xT")
    wqT_sb = load(const, wqT, [P, NH * P], BF16, "wqT")
    ropeC_sb = load(const, ropeC, [P, S], F32, "ropeC", nc.scalar)
    ropeS2_sb = load(const, ropeS2, [P, S], F32, "ropeS2", nc.scalar)
    wkT_sb = load(const, wkT, [P, NH * P], BF16, "wkT", nc.scalar)
    wvT_sb = load(const, wvT, [P, NH * P], BF16, "wvT", nc.scalar)
    tri_sb = load(const, tri, [P, P], BF16, "tri", nc.scalar)
    idn_sb = load(const, idn, [P, P], BF16, "idn", nc.scalar)
    ones_sb = load(const, ones, [P, P], BF16, "ones", nc.scalar)
    woT_sb = load(const, woT, [P, NH * P], BF16, "woT", nc.scalar)

    # v for all heads, [s_in_tile, s_tile, head*128+d], bf16
    v_sb = vpool.tile([P, NT, NH * P], BF16, tag="v")

    def emit_vproj(tiles):
        for st_i in tiles:
            for c in range(2):
                vp = pp.tile([P, 512], F32, tag="proj", name=f"vp{st_i}_{c}")
                nc.tensor.matmul(vp[:], xT_sb[:, st_i * P:(st_i + 1) * P],
                                 wvT_sb[:, c * 512:(c + 1) * 512],
                                 start=True, stop=True)
                nc.scalar.copy(v_sb[:, st_i, c * 512:(c + 1) * 512], vp[:])

    # persistent output accumulator psum (2 banks)
    out_ps = [op.tile([P, 512], F32, tag="out", name=f"out_ps{c}")
              for c in range(2)]

    qrot = {}
    krot = {}
    ynTs = {}

    def emit_proj_rope(h):
        """Project head h's q/k and apply RoPE -> qrot[h], krot[h].

        The pair-permuted operand is obtained WITHOUT a second projection:
        stg = (S'' (.) q) with S'' = row-pair-permuted ropeS table, then a
        SBUF->SBUF DMA pair-swaps partitions of stg into tmp, so
        rot = C (.) q + tmp needs only one PE projection per tensor.
        """
        wq_c = wqT_sb[:, h * P:(h + 1) * P]
        wk_c = wkT_sb[:, h * P:(h + 1) * P]
        qr = qkpool.tile([P, S], BF16, tag="qrot")
        kr = qkpool.tile([P, S], BF16, tag="krot")
        qtmp = tmppool.tile([P, S], BF16, tag="qtmp")
        ktmp = tmppool.tile([P, S], BF16, tag="ktmp")
        qstg = stgpool.tile([P, S], BF16, tag="qstg")
        kstg = stgpool.tile([P, S], BF16, tag="kstg")

        def proj_pair(wt, dst, tmp, stg):
            aps = []
            for c in range(2):
                a = pp.tile([P, 512], F32, tag="proj", name=f"pa{h}_{c}")
                nc.tensor.matmul(a[:], wt, xT_sb[:, c * 512:(c + 1) * 512],
                                 start=True, stop=True)
                aps.append(a)
            for c in range(2):
                sl = slice(c * 512, (c + 1) * 512)
                nc.vector.tensor_tensor(dst[:, sl], aps[c][:],
                                        ropeC_sb[:, sl], MULT)
                nc.vector.tensor_tensor(stg[:, sl], aps[c][:],
                                        ropeS2_sb[:, sl], MULT)
            # partition pair-swap stg -> tmp via two strided SBUF DMAs
            nc.sync.dma_start(tmp[0::2, :], stg[1::2, :])
            nc.sync.dma_start(tmp[1::2, :], stg[0::2, :])
            # bf16 SBUF-only add on the otherwise-idle GPSIMD engine
            nc.gpsimd.tensor_tensor(dst[:], dst[:], tmp[:], ADD)

        def first_half():
            proj_pair(wq_c, qr, qtmp, qstg)
            qrot[h] = qr

        def second_half():
            proj_pair(wk_c, kr, ktmp, kstg)
            krot[h] = kr

        return first_half, second_half

    # S^T psum-tile grouping: (tile cols, [(j, qstart, qend), ...])
    # Each group becomes one PSUM tile and ONE exp instruction; output goes
    # to packed pT cols OFF[j] + (qstart - j*128) .. which are contiguous
    # across the group's members.
    ST_GROUPS = [
        [(0, 0, 1024)],
        [(1, 128, 1024)],
        [(2, 256, 1024)],
        [(3, 384, 1024)],
        [(4, 512, 1024), (5, 640, 1024)],
        [(6, 768, 1024), (7, 896, 1024)],
    ]

    def emit_st_group(g, group, pT):
        """S^T blocks for one psum group of head g, then one exp."""
        kr = krot[g]
        width = sum(qe - qs for (_, qs, qe) in group)
        stt = sp.tile([P, 1024], F32, tag="att", name=f"st{g}")
        col = 0
        diag_cols = []
        for (j, qs, qe) in group:
            kblk = kr[:, j * P:(j + 1) * P]
            # chunks of <=512 within this j's range; same stationary kblk.
            # First chunk holds the diagonal block: its group is closed by
            # the tri-bias matmul; later chunks are self-contained.
            diag_cols.append(OFF[j])
            a = qs
            while a < qe:
                bnd = min(a + 512, qe)
                nc.tensor.matmul(stt[:, col:col + (bnd - a)], kblk,
                                 qrot[g][:, a:bnd], start=True, stop=True)
                col += bnd - a
                a = bnd
        # one exp for the whole group -> packed pT
        j0, qs0, _ = group[0]
        p0 = OFF[j0] + (qs0 - j0 * P)
        nc.scalar.activation(pT[:, p0:p0 + width], stt[:, :width], Exp,
                             scale=SCALE)
        # causal mask: zero the upper triangle of each diagonal block
        # post-exp with cheap bf16-2x DVE multiplies (saves the per-j
        # tri-bias matmul + its serialized LDWEIGHTS on the PE)
        for off in diag_cols:
            nc.vector.tensor_tensor(pT[:, off:off + P], pT[:, off:off + P],
                                    tri_sb[:], MULT)

    def emit_attention_tail(g, pT):
        """Rowsums, AV, single-divide normalization."""
        # rowsums via all-ones matmul; both 512-halves in one [P,1024] tile;
        # all 12 matmuls share the ones stationary back-to-back.
        rs = sp.tile([P, 1024], F32, tag="att", name=f"rs{g}")
        for c in range(2):
            jmax = 4 * c + 3
            for j in range(jmax + 1):
                r0 = max(c * 512, j * P)
                r1 = (c + 1) * 512
                q0 = OFF[j] + r0 - j * P
                nc.tensor.matmul(rs[:, r0:r1], ones_sb[:],
                                 pT[:, q0:q0 + (r1 - r0)],
                                 start=(j == 0), stop=(j == jmax))
        # y^T = sum_j v_j @ P^T_j  (v stationary per j covers both halves)
        y = sp.tile([P, 1024], F32, tag="att", name=f"y{g}")
        for j in range(NT):
            vblk = v_sb[:, j, g * P:(g + 1) * P]
            for c in range(2):
                jmax = 4 * c + 3
                if j > jmax:
                    continue
                r0 = max(c * 512, j * P)
                r1 = (c + 1) * 512
                q0 = OFF[j] + r0 - j * P
                nc.tensor.matmul(y[:, r0:r1], vblk,
                                 pT[:, q0:q0 + (r1 - r0)],
                                 start=(j == 0), stop=(j == jmax))
        # ynT = y * (1/rowsum): recip PSUM->SBUF, then mult (one PSUM input)
        ri = npool.tile([P, S], F32, tag="ri")
        nc.vector.reciprocal_approx_fast(ri[:], rs[:])
        ynT = npool.tile([P, S], BF16, tag="ynT")
        nc.vector.tensor_tensor(ynT[:], y[:], ri[:], MULT)
        ynTs[g] = ynT

    def emit_outproj(g):
        ynT = ynTs.pop(g)
        for c in range(2):
            nc.tensor.matmul(out_ps[c][:], woT_sb[:, g * P:(g + 1) * P],
                             ynT[:, c * 512:(c + 1) * 512],
                             start=(g == 0), stop=(g == NH - 1))

    # software-pipelined head loop (same skeleton as baseline)
    halves = {}
    pTs = {}
    for it in range(NH + 2):
        if it < NH:
            halves[it] = emit_proj_rope(it)
            halves[it][0]()  # q projections + rope
        if 1 <= it <= NH:
            g = it - 1
            pTs[g] = ppool.tile([P, PTW], BF16, tag="pT", name=f"pT{g}")
            for grp in ST_GROUPS[:4]:
                emit_st_group(g, grp, pTs[g])
        if it >= 2:
            emit_outproj(it - 2)
        if it < NH:
            halves[it][1]()  # k projections + rope
        if it == 0:
            emit_vproj(range(NT))
        if 1 <= it <= NH:
            g = it - 1
            for grp in ST_GROUPS[4:]:
                emit_st_group(g, grp, pTs[g])
            emit_attention_tail(g, pTs.pop(g))
            qrot.pop(g), krot.pop(g)

    # output: PSUM -> SBUF -> HBM (DMA cannot read PSUM)
    out_sb = npool.tile([P, S], F32, tag="osb")
    for c in range(2):
        nc.scalar.copy(out_sb[:, c * 512:(c + 1) * 512], out_ps[c][:])
    nc.sync.dma_start(outT, out_sb[:])
    ctx.close()


def _rope_tables_np():
    """Bit-faithful replication of reference._rope_tables (float32 jax ops)."""
    import jax.numpy as jnp
    half = E // 2
    dtype = jnp.float32
    angles = jnp.power(jnp.asarray(10000.0, dtype),
                       2.0 * jnp.arange(half, dtype=dtype) / E)
    theta = jnp.arange(S, dtype=dtype)[:, None] * angles[None, :]
    return np.asarray(jnp.cos(theta)), np.asarray(jnp.sin(theta))


def make_in_maps(x, w_q, w_k, w_v, w_o):
    x = np.asarray(x, np.float32)
    w_q = np.asarray(w_q, np.float32)
    w_k = np.asarray(w_k, np.float32)
    w_v = np.asarray(w_v, np.float32)
    w_o = np.asarray(w_o, np.float32)

    cos, sin = _rope_tables_np()            # [S, 64] f32
    ropeC = np.repeat(cos.T, 2, axis=0)     # [128, S]
    ropeS = np.repeat(sin.T, 2, axis=0)
    ropeS[0::2] *= -1.0
    ropeC = np.ascontiguousarray(ropeC, np.float32)
    # row-pair-permuted ropeS: stg = q (.) ropeS2, then partition pair-swap
    # of stg equals ropeS (.) perm(q)
    ropeS2 = np.ascontiguousarray(ropeS[np.arange(P) ^ 1], np.float32)

    BF = ml_dtypes.bfloat16
    # 0/1 causal mask for a [k_within, q_within] diagonal block: keep k <= q
    tri = np.where(np.arange(P)[None, :] < np.arange(P)[:, None],
                   np.float32(0.0), np.float32(1.0)).astype(BF)
    idn = np.eye(P, dtype=np.float32).astype(BF)

    perm = np.arange(P) ^ 1  # swap adjacent pairs

    def blocksT(w, heads, permute=False):
        cols = []
        for hgl in heads:
            blk = w[hgl * P:(hgl + 1) * P, :]
            if permute:
                blk = blk[perm, :]
            cols.append(blk.T)
        return np.ascontiguousarray(np.concatenate(cols, axis=1)).astype(BF)

    in_maps = []
    for core in range(NCORES):
        b = core // 2
        g = core % 2
        heads = [g * NH + j for j in range(NH)]
        woTc = np.concatenate(
            [w_o[:, h * P:(h + 1) * P].T for h in heads], axis=1)
        in_maps.append({
            "xT": np.ascontiguousarray(x[b].T).astype(BF),
            "wqT": blocksT(w_q, heads),
            "wkT": blocksT(w_k, heads),
            "wvT": blocksT(w_v, heads),
            "woT": np.ascontiguousarray(woTc).astype(BF),
            "ropeC": ropeC,
            "ropeS2": ropeS2,
            "tri": tri,
            "idn": idn,
            "ones": np.ones((P, P), BF),
        })
    return in_maps


_NC_CACHE = {}


def get_nc():
    if "nc" not in _NC_CACHE:
        _NC_CACHE["nc"] = build_bass()
    return _NC_CACHE["nc"]


def run(x, w_q, w_k, w_v, w_o, trace=False, trace_cores=None):
    nc = get_nc()
    in_maps = make_in_maps(x, w_q, w_k, w_v, w_o)
    res = run_bass_kernel_spmd(nc, in_maps, list(range(NCORES)), trace=trace,
                               trace_cores=trace_cores)
    out = np.zeros((B, S, E), np.float32)
    for core in range(NCORES):
        out[core // 2] += res.results[core]["outT"].T
    return out, res


def kernel(x, w_q, w_k, w_v, w_o):
    out, _ = run(x, w_q, w_k, w_v, w_o)
    return out


# revision 48
# speedup vs baseline: 1.0851x; 1.0851x over previous
"""Multi-head attention (16 heads, RoPE, causal) Trainium2 Bass kernel.

Sharding: 8 cores = 4-way data-parallel over batch x 2-way tensor-parallel
over heads (each core: 1 batch, 8 heads). Per-core partial outputs (over its
8 heads) are summed pairwise on the host (the w_o "all-reduce").

v1 design (all-bf16 matmul path):
  - All matmul operands bf16 (stationaries get Fast Weight Load, no fp32r
    narrow-moving penalty, half SBUF traffic). PSUM accumulation is fp32.
  - x passed pre-transposed xT [e, s]; per-head w blocks [e, d] so
    projections produce qT/kT [d, s] directly.
  - RoPE: qrot = C (.) q + S' (.) qp with qp from pair-permuted weight
    copies; the two multiplies run on DVE (PSUM fp32 in, bf16 out), the
    add runs on DVE in bf16 2x mode (one [P,1024] instr per tensor).
  - S^T[k, q] per k-tile j lands in 2-bank [P, <=1024] PSUM tiles grouped
    so ONE Scalar-engine exp covers each tile (6 exps/head instead of 12):
    tiles j=0..3 alone, (j=4,5) share, (j=6,7) share. Causal diagonal gets
    a -1e30 upper-tri bias via one bf16 idn x tri matmul per j.
  - P^T stored PACKED by triangle: pT[:, off_j + (q - j*128)] with
    off_j = j*1024 - 64*j*(j-1), total 4608 cols (bf16).
  - rowsums: 12 ones-matmuls (same stationary back to back) into one
    [P, 1024] PSUM tile; AV into another; normalization = single GPSIMD
    divide per head ([P,1024], PSUM/PSUM -> bf16 SBUF) - no reciprocal.
  - v for all heads projected once (x-tile stationary), copied PSUM->SBUF
    bf16 on GPSIMD.
  - out^T accumulated over heads in 2 persistent PSUM banks, DMA'd
    directly PSUM->HBM.
"""

import os
import sys

import ml_dtypes
import numpy as np

for _p in ("/opt/trn_rl_repo",):
    if os.path.isdir(_p) and _p not in sys.path:
        sys.path.append(_p)

import concourse.bass as bass  # noqa: E402
import concourse.tile as tile  # noqa: E402
from concourse import bacc, mybir  # noqa: E402
import concourse.bass_utils as _bass_utils  # noqa: E402
from concourse.bass_utils import run_bass_kernel_spmd  # noqa: E402



F32 = mybir.dt.float32
BF16 = mybir.dt.bfloat16

B, S, E, H = 4, 1024, 128, 16
NCORES = 8
NH = 8          # heads per core
P = 128
NT = S // P     # 8 seq tiles
SCALE = 1.0 / float(np.sqrt(np.float32(E)))
Exp = mybir.ActivationFunctionType.Exp
MULT = mybir.AluOpType.mult
ADD = mybir.AluOpType.add
DIV = mybir.AluOpType.divide

# packed-triangle column offsets: off[j] = sum_{i<j} (S - i*P)
OFF = [0]
for _j in range(NT):
    OFF.append(OFF[-1] + S - _j * P)
PTW = OFF[NT]   # 4608 packed columns per head


def build_bass():
    nc = bacc.Bacc("TRN2", target_bir_lowering=False, debug=False,
                   num_devices=NCORES)

    def din(name, shape, dt=BF16):
        return nc.dram_tensor(name, shape, dt, kind="ExternalInput").ap()

    xT = din("xT", [P, S])
    wqT = din("wqT", [P, NH * P])
    wkT = din("wkT", [P, NH * P])
    wvT = din("wvT", [P, NH * P])
    woT = din("woT", [P, NH * P])
    ropeC = din("ropeC", [P, S], F32)
    ropeS2 = din("ropeS2", [P, S], F32)
    tri = din("tri", [P, P])
    idn = din("idn", [P, P])
    ones = din("ones", [P, P])
    outT = nc.dram_tensor("outT", [P, S], F32, kind="ExternalOutput").ap()

    with tile.TileContext(nc) as tc:
        _build(tc, xT, wqT, wkT, wvT, woT, ropeC, ropeS2, tri,
               idn, ones, outT)
    _dedup_ldweights(nc)
    nc.compile()
    return nc


def _dedup_ldweights(nc):
    """Remove redundant PE weight loads.

    Legalization inserts one InstLdweights per InstMatmult; the hardware
    keeps the stationary operand in the array across matmuls, so a load
    identical to the previous one on the PE queue is pure overhead
    (~88ns serialized each). Only sync-free loads are removed, and only
    when no different load intervenes.
    """
    removed = 0
    for f in nc.m.functions:
        for b in f.blocks:
            last_key = None
            keep = []
            for inst in b.instructions:
                if isinstance(inst, mybir.InstLdweights):
                    si = inst.sync_info
                    clean = si is None or (not si.on_wait and
                                           not si.on_update)
                    w = inst.ins[0]
                    key = (repr(w), getattr(inst, "perf_mode", None),
                           getattr(inst, "is_transpose", None))
                    if clean and key == last_key:
                        removed += 1
                        continue
                    last_key = key
                keep.append(inst)
            b.instructions[:] = keep
    return removed


def _build(tc, xT, wqT, wkT, wvT, woT, ropeC, ropeS2, tri, idn,
           ones, outT):
    nc = tc.nc

    from contextlib import ExitStack
    ctx = ExitStack()
    const = ctx.enter_context(tc.tile_pool(name="const", bufs=1))
    vpool = ctx.enter_context(tc.tile_pool(name="vpool", bufs=1))
    ppool = ctx.enter_context(tc.tile_pool(name="ppool", bufs=3))
    qkpool = ctx.enter_context(tc.tile_pool(name="qkpool", bufs=6))
    tmppool = ctx.enter_context(tc.tile_pool(name="tmppool", bufs=3))
    stgpool = ctx.enter_context(tc.tile_pool(name="stgpool", bufs=3))
    npool = ctx.enter_context(tc.tile_pool(name="npool", bufs=2))
    pp = ctx.enter_context(tc.tile_pool(name="pp", bufs=2, space="PSUM"))
    sp = ctx.enter_context(tc.tile_pool(name="sp", bufs=2, space="PSUM"))
    op = ctx.enter_context(tc.tile_pool(name="op", bufs=2, space="PSUM"))

    # ---- constants into SBUF in first-use order, spread across the idle
    # engine DMA queues so issue time (~0.6us each) isn't serialized on SP
    def load(pool, ap, shape, dt, tag, eng=None):
        t = pool.tile(shape, dt, tag=tag)
        (eng or nc.sync).dma_start(t[:], ap)
        return t

    xT_sb = load(const, xT, [P, S], BF16, "xT")
    wqT_sb = load(const, wqT, [P, NH * P], BF16, "wqT")
    ropeC_sb = load(const, ropeC, [P, S], F32, "ropeC", nc.scalar)
    ropeS2_sb = load(const, ropeS2, [P, S], F32, "ropeS2", nc.scalar)
    wkT_sb = load(const, wkT, [P, NH * P], BF16, "wkT", nc.scalar)
    wvT_sb = load(const, wvT, [P, NH * P], BF16, "wvT", nc.scalar)
    tri_sb = load(const, tri, [P, P], BF16, "tri", nc.scalar)
    idn_sb = load(const, idn, [P, P], BF16, "idn", nc.scalar)
    ones_sb = load(const, ones, [P, P], BF16, "ones", nc.scalar)
    woT_sb = load(const, woT, [P, NH * P], BF16, "woT", nc.scalar)

    # v for all heads, [s_in_tile, s_tile, head*128+d], bf16
    v_sb = vpool.tile([P, NT, NH * P], BF16, tag="v")

    def emit_vproj(tiles):
        for st_i in tiles:
            for c in range(2):
                vp = pp.tile([P, 512], F32, tag="proj", name=f"vp{st_i}_{c}")
                nc.tensor.matmul(vp[:], xT_sb[:, st_i * P:(st_i + 1) * P],
                                 wvT_sb[:, c * 512:(c + 1) * 512],
                                 start=True, stop=True)
                nc.scalar.copy(v_sb[:, st_i, c * 512:(c + 1) * 512], vp[:])

    # persistent output accumulator psum (2 banks)
    out_ps = [op.tile([P, 512], F32, tag="out", name=f"out_ps{c}")
              for c in range(2)]

    qrot = {}
    krot = {}
    ynTs = {}

    def emit_proj_rope(h):
        """Project head h's q/k and apply RoPE -> qrot[h], krot[h].

        The pair-permuted operand is obtained WITHOUT a second projection:
        stg = (S'' (.) q) with S'' = row-pair-permuted ropeS table, then a
        SBUF->SBUF DMA pair-swaps partitions of stg into tmp, so
        rot = C (.) q + tmp needs only one PE projection per tensor.
        """
        wq_c = wqT_sb[:, h * P:(h + 1) * P]
        wk_c = wkT_sb[:, h * P:(h + 1) * P]
        qr = qkpool.tile([P, S], BF16, tag="qrot")
        kr = qkpool.tile([P, S], BF16, tag="krot")
        qtmp = tmppool.tile([P, S], BF16, tag="qtmp")
        ktmp = tmppool.tile([P, S], BF16, tag="ktmp")
        qstg = stgpool.tile([P, S], BF16, tag="qstg")
        kstg = stgpool.tile([P, S], BF16, tag="kstg")

        def proj_pair(wt, dst, tmp, stg):
            aps = []
            for c in range(2):
                a = pp.tile([P, 512], F32, tag="proj", name=f"pa{h}_{c}")
                nc.tensor.matmul(a[:], wt, xT_sb[:, c * 512:(c + 1) * 512],
                                 start=True, stop=True)
                aps.append(a)
            for c in range(2):
                sl = slice(c * 512, (c + 1) * 512)
                nc.vector.tensor_tensor(dst[:, sl], aps[c][:],
                                        ropeC_sb[:, sl], MULT)
                nc.vector.tensor_tensor(stg[:, sl], aps[c][:],
                                        ropeS2_sb[:, sl], MULT)
            # partition pair-swap stg -> tmp via two strided SBUF DMAs
            nc.sync.dma_start(tmp[0::2, :], stg[1::2, :])
            nc.sync.dma_start(tmp[1::2, :], stg[0::2, :])
            # bf16 SBUF-only add on the otherwise-idle GPSIMD engine
            nc.gpsimd.tensor_tensor(dst[:], dst[:], tmp[:], ADD)

        def first_half():
            proj_pair(wq_c, qr, qtmp, qstg)
            qrot[h] = qr

        def second_half():
            proj_pair(wk_c, kr, ktmp, kstg)
            krot[h] = kr

        return first_half, second_half

    # S^T psum-tile grouping: (tile cols, [(j, qstart, qend), ...])
    # Each group becomes one PSUM tile and ONE exp instruction; output goes
    # to packed pT cols OFF[j] + (qstart - j*128) .. which are contiguous
    # across the group's members.
    ST_GROUPS = [
        [(0, 0, 1024)],
        [(1, 128, 1024)],
        [(2, 256, 1024)],
        [(3, 384, 1024)],
        [(4, 512, 1024), (5, 640, 1024)],
        [(6, 768, 1024), (7, 896, 1024)],
    ]

    def emit_st_group(g, group, pT):
        """S^T blocks for one psum group of head g, then one exp."""
        kr = krot[g]
        width = sum(qe - qs for (_, qs, qe) in group)
        stt = sp.tile([P, 1024], F32, tag="att", name=f"st{g}")
        col = 0
        diag_cols = []
        for (j, qs, qe) in group:
            kblk = kr[:, j * P:(j + 1) * P]
            # chunks of <=512 within this j's range; same stationary kblk.
            # First chunk holds the diagonal block: its group is closed by
            # the tri-bias matmul; later chunks are self-contained.
            diag_cols.append(col)
            a = qs
            while a < qe:
                bnd = min(a + 512, qe)
                nc.tensor.matmul(stt[:, col:col + (bnd - a)], kblk,
                                 qrot[g][:, a:bnd], start=True,
                                 stop=(a != qs))
                col += bnd - a
                a = bnd
        # causal tri bias into each j's diagonal block (idn stationary).
        # Post-exp masking on DVE/GPSIMD was tried and is SLOWER: the mask
        # serializes rowsum/AV behind an elementwise engine; the PE matmul
        # keeps the dependency chain inside the PE queue.
        for c0 in diag_cols:
            nc.tensor.matmul(stt[:, c0:c0 + P], idn_sb[:], tri_sb[:],
                             start=False, stop=True)
        # one exp for the whole group -> packed pT
        j0, qs0, _ = group[0]
        p0 = OFF[j0] + (qs0 - j0 * P)
        nc.scalar.activation(pT[:, p0:p0 + width], stt[:, :width], Exp,
                             scale=SCALE)

    def emit_attention_tail(g, pT):
        """Rowsums, AV, single-divide normalization."""
        # rowsums via all-ones matmul; both 512-halves in one [P,1024] tile;
        # all 12 matmuls share the ones stationary back-to-back.
        rs = sp.tile([P, 1024], F32, tag="att", name=f"rs{g}")
        for c in range(2):
            jmax = 4 * c + 3
            for j in range(jmax + 1):
                r0 = max(c * 512, j * P)
                r1 = (c + 1) * 512
                q0 = OFF[j] + r0 - j * P
                nc.tensor.matmul(rs[:, r0:r1], ones_sb[:],
                                 pT[:, q0:q0 + (r1 - r0)],
                                 start=(j == 0), stop=(j == jmax))
        # y^T = sum_j v_j @ P^T_j  (v stationary per j covers both halves)
        y = sp.tile([P, 1024], F32, tag="att", name=f"y{g}")
        for j in range(NT):
            vblk = v_sb[:, j, g * P:(g + 1) * P]
            for c in range(2):
                jmax = 4 * c + 3
                if j > jmax:
                    continue
                r0 = max(c * 512, j * P)
                r1 = (c + 1) * 512
                q0 = OFF[j] + r0 - j * P
                nc.tensor.matmul(y[:, r0:r1], vblk,
                                 pT[:, q0:q0 + (r1 - r0)],
                                 start=(j == 0), stop=(j == jmax))
        # ynT = y * (1/rowsum): recip PSUM->SBUF, then mult (one PSUM input)
        ri = npool.tile([P, S], F32, tag="ri")
        nc.vector.reciprocal_approx_fast(ri[:], rs[:])
        ynT = npool.tile([P, S], BF16, tag="ynT")
        nc.vector.tensor_tensor(ynT[:], y[:], ri[:], MULT)
        ynTs[g] = ynT

    def emit_outproj(g):
        ynT = ynTs.pop(g)
        for c in range(2):
            nc.tensor.matmul(out_ps[c][:], woT_sb[:, g * P:(g + 1) * P],
                             ynT[:, c * 512:(c + 1) * 512],
                             start=(g == 0), stop=(g == NH - 1))

    # software-pipelined head loop (same skeleton as baseline)
    halves = {}
    pTs = {}
    for it in range(NH + 2):
        if it < NH:
            halves[it] = emit_proj_rope(it)
            halves[it][0]()  # q projections + rope
        if 1 <= it <= NH:
            g = it - 1
            pTs[g] = ppool.tile([P, PTW], BF16, tag="pT", name=f"pT{g}")
            for grp in ST_GROUPS[:4]:
                emit_st_group(g, grp, pTs[g])
        if it >= 2:
            emit_outproj(it - 2)
        if it < NH:
            halves[it][1]()  # k projections + rope
        if it == 0:
            emit_vproj(range(NT))
        if 1 <= it <= NH:
            g = it - 1
            for grp in ST_GROUPS[4:]:
                emit_st_group(g, grp, pTs[g])
            emit_attention_tail(g, pTs.pop(g))
            qrot.pop(g), krot.pop(g)

    # output: PSUM -> SBUF -> HBM (DMA cannot read PSUM)
    out_sb = npool.tile([P, S], F32, tag="osb")
    for c in range(2):
        nc.scalar.copy(out_sb[:, c * 512:(c + 1) * 512], out_ps[c][:])
    nc.sync.dma_start(outT, out_sb[:])
    ctx.close()


def _rope_tables_np():
    """Bit-faithful replication of reference._rope_tables (float32 jax ops)."""
    import jax.numpy as jnp
    half = E // 2
    dtype = jnp.float32
    angles = jnp.power(jnp.asarray(10000.0, dtype),
                       2.0 * jnp.arange(half, dtype=dtype) / E)
    theta = jnp.arange(S, dtype=dtype)[:, None] * angles[None, :]
    return np.asarray(jnp.cos(theta)), np.asarray(jnp.sin(theta))


def make_in_maps(x, w_q, w_k, w_v, w_o):
    x = np.asarray(x, np.float32)
    w_q = np.asarray(w_q, np.float32)
    w_k = np.asarray(w_k, np.float32)
    w_v = np.asarray(w_v, np.float32)
    w_o = np.asarray(w_o, np.float32)

    cos, sin = _rope_tables_np()            # [S, 64] f32
    ropeC = np.repeat(cos.T, 2, axis=0)     # [128, S]
    ropeS = np.repeat(sin.T, 2, axis=0)
    ropeS[0::2] *= -1.0
    ropeC = np.ascontiguousarray(ropeC, np.float32)
    # row-pair-permuted ropeS: stg = q (.) ropeS2, then partition pair-swap
    # of stg equals ropeS (.) perm(q)
    ropeS2 = np.ascontiguousarray(ropeS[np.arange(P) ^ 1], np.float32)

    BF = ml_dtypes.bfloat16
    tri = np.where(np.arange(P)[None, :] < np.arange(P)[:, None],
                   np.float32(-1e30), np.float32(0.0)).astype(BF)
    idn = np.eye(P, dtype=np.float32).astype(BF)

    perm = np.arange(P) ^ 1  # swap adjacent pairs

    def blocksT(w, heads, permute=False):
        cols = []
        for hgl in heads:
            blk = w[hgl * P:(hgl + 1) * P, :]
            if permute:
                blk = blk[perm, :]
            cols.append(blk.T)
        return np.ascontiguousarray(np.concatenate(cols, axis=1)).astype(BF)

    in_maps = []
    for core in range(NCORES):
        b = core // 2
        g = core % 2
        heads = [g * NH + j for j in range(NH)]
        woTc = np.concatenate(
            [w_o[:, h * P:(h + 1) * P].T for h in heads], axis=1)
        in_maps.append({
            "xT": np.ascontiguousarray(x[b].T).astype(BF),
            "wqT": blocksT(w_q, heads),
            "wkT": blocksT(w_k, heads),
            "wvT": blocksT(w_v, heads),
            "woT": np.ascontiguousarray(woTc).astype(BF),
            "ropeC": ropeC,
            "ropeS2": ropeS2,
            "tri": tri,
            "idn": idn,
            "ones": np.ones((P, P), BF),
        })
    return in_maps


_NC_CACHE = {}


def get_nc():
    if "nc" not in _NC_CACHE:
        _NC_CACHE["nc"] = build_bass()
    return _NC_CACHE["nc"]


def run(x, w_q, w_k, w_v, w_o, trace=False, trace_cores=None):
    nc = get_nc()
    in_maps = make_in_maps(x, w_q, w_k, w_v, w_o)
    res = run_bass_kernel_spmd(nc, in_maps, list(range(NCORES)), trace=trace,
                               trace_cores=trace_cores)
    out = np.zeros((B, S, E), np.float32)
    for core in range(NCORES):
        out[core // 2] += res.results[core]["outT"].T
    return out, res


def kernel(x, w_q, w_k, w_v, w_o):
    out, _ = run(x, w_q, w_k, w_v, w_o)
    return out
